# revision 15
# baseline (speedup 1.0000x reference)
"""Valid 3x3x3 conv3d: x[2,32,64,64,64] (*) W[64,32,3,3,3] -> y[2,64,62,62,62].

Sharding: D axis split across 8 cores (8 output planes each, 2-plane input halo,
sliced host-side). Batch = 2 independent streams per core (double-buffered SBUF
slots) so plane DMA+rounding overlaps PE compute of the other stream.

Per-core compute: conv as 27 shifted matmuls reduced to 6 per 8-row block:
  - K = 96: in_c(32) x kz(3); plane d lives at partition group (d mod 3), so the
    3 kz taps of any output plane occupy disjoint partition groups with no data
    replication. Weight column layout is rotated per (output plane mod 3).
  - kx taps 0,1 are paired into one M=128 matmul (rows 0:64 aligned, rows
    64:128 misaligned by +1 output column, fixed in the ACT+DVE combine); kx=2
    is an aligned M=64 matmul. 3 ky taps -> 3 pairs + 3 singles, all
    accumulating into one PSUM bank per 8-row block (N = nh*64 <= 512).
  - fp32r matmuls (1 col/cycle at N>=256, ~1.4e-4 rel err); inputs DMA straight
    into fp32r SBUF tiles (walrus accepts DMACopy as the fp32r producer; all
    fp32r APs must have even innermost counts/strides and dst partition 0).
"""
import sys
sys.path.insert(0, '/opt/trn_rl_repo')
import numpy as np

IN_C, OUT_C = 32, 64
SH = SW = 64
OD = 62
PD = 8          # output planes per core per batch
HALO = 2
NB = 2          # batches/streams
BLOCKS = [(h0, 8 if h0 + 8 <= OD else OD - h0) for h0 in range(0, OD, 8)]

_cache = {}


def _build():
    import concourse.bacc as bacc
    import concourse.mybir as mybir
    from concourse import tile
    dt = mybir.dt

    nc = bacc.Bacc(trn_type="TRN2")
    x_d = nc.declare_dram_parameter("x", [NB, IN_C, PD + HALO, SH * SW],
                                    dt.float32, isOutput=False)
    w_d = nc.declare_dram_parameter("w", [96, 3, 3, 192], dt.float32,
                                    isOutput=False)
    y_d = nc.declare_dram_parameter("y", [NB, OUT_C, PD, OD, OD], dt.float32,
                                    isOutput=True)

    with tile.TileContext(nc) as tc:
        with tc.tile_pool(name="xb", bufs=1) as xb_pool, \
             tc.tile_pool(name="wb", bufs=1) as wb_pool, \
             tc.tile_pool(name="ps", bufs=4, space="PSUM") as ps_pool, \
             tc.tile_pool(name="ob", bufs=4) as ob_pool:

            # weights: DMA straight into fp32r (bit-identical; walrus accepts
            # DMACopy as an fp32r producer)
            wbuf = wb_pool.tile([96, 3, 3, 192], dt.float32r)
            nc.sync.dma_start(out=wbuf[:, :, :, :],
                              in_=w_d[:, :, :, :].bitcast(dt.float32r))

            # x planes: persistent fp32r buffer, 2 stream slots, cyclic-3 groups
            xbuf = xb_pool.tile([128, NB, SH * SW + 4], dt.float32r)

            def load_plane(s, dz):
                g = dz % 3
                nc.sync.dma_start(out=xbuf[g * 32:(g + 1) * 32, s, 0:SH * SW],
                                  in_=x_d[s, :, dz, :].bitcast(dt.float32r))

            def compute_plane(s, k):
                r = k % 3
                for h0, nh in BLOCKS:
                    p = ps_pool.tile([128, 8, 64], dt.float32)
                    first = True
                    for ky in range(3):  # pairs (kx0|kx1), M=128, offset 0
                        off = (h0 + ky) * 64
                        nc.tensor.matmul(
                            p[:, 0:nh, :],
                            wbuf[:, r, ky, 0:128],
                            xbuf[0:96, s, off:off + nh * 64],
                            start=first, stop=False)
                        first = False
                    for ky in range(3):  # singles kx2, M=64, offset 2
                        off = (h0 + ky) * 64 + 2
                        nc.tensor.matmul(
                            p[0:64, 0:nh, :],
                            wbuf[:, r, ky, 128:192],
                            xbuf[0:96, s, off:off + nh * 64],
                            start=False, stop=(ky == 2))
                    t2 = ob_pool.tile([64, 8, 62], dt.float32, tag="shift")
                    nc.scalar.copy(t2[:, 0:nh, :], p[64:128, 0:nh, 1:63])
                    o = ob_pool.tile([64, 8, 62], dt.float32)
                    nc.vector.tensor_add(o[:, 0:nh, :], p[0:64, 0:nh, 0:62],
                                         t2[:, 0:nh, :])
                    nc.sync.dma_start(out=y_d[s, :, k, h0:h0 + nh, :],
                                      in_=o[:, 0:nh, :])

            for s in range(NB):
                for dz in range(3):
                    load_plane(s, dz)
            for k in range(PD):
                for s in range(NB):
                    compute_plane(s, k)
                    if k + 3 < PD + HALO:
                        load_plane(s, k + 3)

    nc.compile()
    return nc


def _weights_rot(Wf):
    """[96, 3(rot), 3(ky), 192] with cols [kx1 | kx2 | kx0], kz=(g-r)%3."""
    Wr = np.zeros((96, 3, 3, 192), np.float32)
    for r in range(3):
        for g in range(3):
            kz = (g - r) % 3
            for ky in range(3):
                blk = Wf[:, :, kz, ky, :]  # [oc, ic, kx]
                sl = slice(g * 32, (g + 1) * 32)
                Wr[sl, r, ky, 0:64] = blk[:, :, 0].T
                Wr[sl, r, ky, 64:128] = blk[:, :, 1].T
                Wr[sl, r, ky, 128:192] = blk[:, :, 2].T
    return Wr


def kernel(x, W):
    from concourse.bass_utils import run_bass_kernel_spmd
    x = np.ascontiguousarray(np.asarray(x), np.float32)
    W = np.ascontiguousarray(np.asarray(W), np.float32)
    if "nc" not in _cache:
        _cache["nc"] = _build()
    nc = _cache["nc"]

    xp = np.zeros((NB, IN_C, 8 * PD + HALO, SH, SW), np.float32)
    xp[:, :, :64] = x
    Wr = _weights_rot(W)
    xpf = xp.reshape(NB, IN_C, 8 * PD + HALO, SH * SW)
    in_maps = [{"x": np.ascontiguousarray(xpf[:, :, c * PD:c * PD + PD + HALO]),
                "w": Wr} for c in range(8)]
    res = run_bass_kernel_spmd(nc, in_maps, core_ids=list(range(8)))

    out = np.empty((NB, OUT_C, OD, OD, OD), np.float32)
    for c in range(8):
        lo = c * PD
        n = min(PD, OD - lo)
        if n > 0:
            out[:, :, lo:lo + n] = res.results[c]["y"][:, :, :n]
    return out
